# revision 20
# baseline (speedup 1.0000x reference)
"""Bloom self-attention (fused QKV + causal softmax attention) on 8 TRN2 cores.

Sharding: core c handles batch b=c//2 and head-group hg=c%2 (8 of 16 heads).
Each core computes QKV projection for its columns of W, then causal attention
for its 8 heads, writing out[s, 1024] (fp32). Host transposes/casts/slices
inputs and gathers outputs.

QKV projection runs in fp8 (e4m3) DoubleRow mode with a 3-term residual
decomposition: X = Xh + Xl, W = Wh + Wl (each fp8 hi + fp8 residual of the
fp32 value), and X@W ~= Xh@Wh + Xh@Wl + Xl@Wh. DoubleRow contracts two
128-deep subtiles per instruction at 0.5 cycles/row, so the 3 products cost
0.75x the bf16 equivalent while keeping ~bf16 accuracy:
  instr1[d]: lhsT=(Wh_d,Wl_d) contiguous pair, rhs=Xh_d broadcast (stride-0)
             -> Wh_d.T Xh_d + Wl_d.T Xh_d
  instr2[d,d+1]: lhsT=(Wh_d,Wh_d+1) stride-2, rhs=(Xl_d,Xl_d+1) stride-2
             -> Wh_d.T Xl_d + Wh_d+1.T Xl_d+1
W is pre-scaled by 32 on host so its entries (~N(0, 1/2048)) sit in fp8's
normal range; the 32x on q,k is folded into the exp scale (/32/32), and the
32x on v is folded into the rowsum-normalization by making the interleaved
"ones" columns 32.0 (biases are pre-scaled by 32 to match).

DMAs are batched (multi-subtile xt/wv groups, fused q+k weight loads, one
output DMA per (head, q-block) via a j-interleaved dram view) because the
HWDGE descriptor generator is a serial ~630ns/DMA resource that otherwise
starves the PE during the startup ramp.

Layout notes (per core, on device):
  xt8  [128,32,2048] fp8  : X_b^T d-subtiles, (hi_d, lo_d) interleaved
  wqk  [8,128,64,128] fp8 : per head, q then k (hi_d, lo_d) W subtiles
  wv   [2,128,32,512] fp8 : per head-quad, 4 heads' v columns batched
  bqk  [128,16]       f32 : per-partition bias columns per head (x32), q|k
  bvq  [2,128,520]   bf16 : v-bias rows (x32) + interleaved 32.0 columns
                            (130-stride: per quad-head 128 v cols, a 32.0 col,
                             a zero pad col) -> attn@[v|32] yields 32*rowsums
  mask [128,896]     bf16 : causal staircase; slice [:, 384-r:384-r+512] is
                            the 0/1 mask for a diagonal block at offset r
  qT/kT[128,2048]    bf16 : hold 32*q, 32*k

The emission order software-pipelines PE-dense work (QKV/V matmul chunks)
against ACT-bound attention chunks of an already-projected head; the last
three heads run "JIT" (projection chunk sb emitted just before the score
chunks that need it) so no step is left with attention-only PE work.
"""

import math
from contextlib import ExitStack

import numpy as np
import ml_dtypes

import concourse.mybir as mybir
import concourse.tile as tile
from concourse import bacc
from concourse.bass_utils import run_bass_kernel_spmd

B, S, D = 4, 2048, 2048
H, HD = 16, 128
N_CORES = 8
DT = D // 128   # 16 d-subtiles
NQB = S // 512  # 4 q-blocks
SCALE = 1.0 / math.sqrt(HD)
WS = 32.0       # host-side W pre-scale

BF16 = mybir.dt.bfloat16
F32 = mybir.dt.float32
FP8 = mybir.dt.float8e4
DR = mybir.MatmulPerfMode.DoubleRow
BF16_NP = ml_dtypes.bfloat16
FP8_NP = ml_dtypes.float8_e4m3fn


def build_nc(repeat: int = 1):
    nc = bacc.Bacc(
        "TRN2",
        target_bir_lowering=False,
        debug=False,
        enable_asserts=False,
        num_devices=N_CORES,
    )
    xt_d = nc.dram_tensor("xt", [2 * DT, 128, S], FP8, kind="ExternalInput")
    wqk_d = nc.dram_tensor("wqk", [8, 128, 2 * 2 * DT * 128], FP8,
                           kind="ExternalInput")
    wv_d = nc.dram_tensor("wv", [2, 128, 2 * DT * 512], FP8, kind="ExternalInput")
    bqk_d = nc.dram_tensor("bqk", [128, 16], F32, kind="ExternalInput")
    bvq_d = nc.dram_tensor("bvq", [2, 128, 520], BF16, kind="ExternalInput")
    mask_d = nc.dram_tensor("mask", [128, 896], BF16, kind="ExternalInput")
    out_d = nc.dram_tensor("out", [S, 1024], F32, kind="ExternalOutput")

    with ExitStack() as ctx:
        tc = ctx.enter_context(tile.TileContext(nc))
        singles = ctx.enter_context(tc.tile_pool(name="singles", bufs=1))
        wqk_pool = ctx.enter_context(tc.tile_pool(name="wqk", bufs=2))
        wv_pool = ctx.enter_context(tc.tile_pool(name="wvp", bufs=2))
        qk_pool = ctx.enter_context(tc.tile_pool(name="qk", bufs=2))
        v4_pool = ctx.enter_context(tc.tile_pool(name="v4", bufs=2))
        p_pool = ctx.enter_context(tc.tile_pool(name="pp", bufs=28))
        o_pool = ctx.enter_context(tc.tile_pool(name="op", bufs=3))
        r_pool = ctx.enter_context(tc.tile_pool(name="rp", bufs=8))
        # Separate PSUM pools: projection matmuls must never queue behind
        # score tiles waiting on ACT exp drain (couples PE to ACT bursts).
        ps_proj = ctx.enter_context(tc.tile_pool(name="ps_proj", bufs=3, space="PSUM"))
        ps_sc = ctx.enter_context(tc.tile_pool(name="ps_sc", bufs=4, space="PSUM"))
        ps_out = ctx.enter_context(tc.tile_pool(name="ps_out", bufs=1, space="PSUM"))

        # ---- resident constants (loaded once) ----
        # Batched startup DMAs, ordered by first consumption: the first
        # v-chunk sweeps d over all wv subtiles but reads only s-quarter 0 of
        # xt, and qkv chunks read xt by s-block too — so xt transfers by
        # s-quarter (not by subtile), with quarter 0 split in half and wv(0)
        # groups interleaved ahead of it. One DMA costs ~630ns of serial
        # HWDGE regardless of size, so batches are as big as pipelining
        # allows.
        wv_first = wv_pool.tile([128, 2 * DT, 512], FP8, tag="wv")
        xt = singles.tile([128, 2 * DT, S], FP8, tag="xt")
        xt_dv = xt_d.ap().rearrange("k p s -> p k s")
        wv_dv = wv_d.ap()[0].rearrange("p (g k n) -> p g k n", g=4, n=512)

        def xt_dma(k0, k1, s0, s1):
            nc.sync.dma_start(
                out=xt[:, k0:k1, s0:s1], in_=xt_dv[:, k0:k1, s0:s1]
            )

        def wv_dma(g):
            nc.sync.dma_start(
                out=wv_first[:, 8 * g : 8 * g + 8, :], in_=wv_dv[:, g]
            )

        # DMA order follows first consumption: v-pass i (st columns 2i,2i+1)
        # reads only s-slice [256i, 256i+256) of xt but sweeps all wv
        # subtiles, so wv front-loads in escalating chunks and xt follows by
        # s-slice. Small constants land early (bvq must precede the first v4
        # finalize or psum-bank reuse stalls the PE; a last-emitted bvq costs
        # a 14us startup stall). Head-0 weights ride along mid-stream so the
        # qkv0 step starts without a DMA wait.
        wv_dmas = wv_d.ap()[0].rearrange("p (k n) -> p k n", n=512)

        def wv_dma2(k0, k1):
            nc.sync.dma_start(out=wv_first[:, k0:k1, :], in_=wv_dmas[:, k0:k1, :])

        wv_dma2(0, 8)
        xt_dma(0, 16, 0, 512)
        bvq = singles.tile([128, 2, 520], BF16, tag="bvq")
        nc.sync.dma_start(out=bvq[:], in_=bvq_d.ap().rearrange("g p c -> p g c"))
        bqk = singles.tile([128, 16], F32, tag="bqk")
        nc.sync.dma_start(out=bqk[:], in_=bqk_d.ap())
        wv_dma2(8, 16)
        xt_dma(16, 32, 0, 512)
        mask = singles.tile([128, 896], BF16, tag="mask")
        nc.sync.dma_start(out=mask[:], in_=mask_d.ap())
        wv_dma2(16, 24)
        wv_dma2(24, 32)
        xt_dma(0, 32, 512, 1024)
        wqk_first = wqk_pool.tile([128, 4 * DT, 128], FP8, tag="wqk")
        nc.sync.dma_start(
            out=wqk_first[:].rearrange("p k m -> p (k m)"), in_=wqk_d.ap()[0]
        )
        xt_dma(0, 32, 1024, 1536)
        xt_dma(0, 32, 1536, 2048)
        # prewarm the ACT exp table set (~2.7us PSEUDO_LOAD on first Exp)
        # while the startup DMAs run, instead of inside the first attention
        # chain
        warm = singles.tile([128, 1], F32, tag="warm")
        nc.vector.memset(warm[:], 0.0)
        nc.scalar.activation(warm[:], warm[:], mybir.ActivationFunctionType.Exp)

        def resid_mms(psx, w8, xs0, n_s):
            """Emit the 24 DoubleRow matmuls for psx[128,n_s] = (X@W).T chunk
            over the full D contraction, for s-columns [xs0, xs0+n_s).
            w8: [128, 2*DT, M] AP with (hi_d, lo_d) interleaved subtiles."""
            wvw = w8.rearrange("p (d two) m -> p two d m", two=2)
            xv = xt[:].rearrange("p (d two) s -> p two d s", two=2)
            for d in range(0, DT, 2):
                for dd in (d, d + 1):
                    nc.tensor.matmul(
                        psx,
                        lhsT=w8[:, 2 * dd : 2 * dd + 2, :],
                        rhs=xt[:, 2 * dd, xs0 : xs0 + n_s]
                        .unsqueeze(1)
                        .to_broadcast([128, 2, n_s]),
                        start=(dd == 0),
                        stop=False,
                        perf_mode=DR,
                    )
                nc.tensor.matmul(
                    psx,
                    lhsT=wvw[:, 0, d : d + 2, :],
                    rhs=xv[:, 1, d : d + 2, xs0 : xs0 + n_s],
                    start=False,
                    stop=(d == DT - 2),
                    perf_mode=DR,
                )

        for _rep in range(repeat):
            # per-rep state: tiles keyed by quad / head
            v4s = {}     # g -> [16 v4 tiles]
            wv_gs = {}   # g -> wv tile
            qks = {}     # h -> (qT, kT, wqk_h)

            def v_start(g):
                if g == 0 and _rep == 0:
                    wv_g = wv_first
                else:
                    wv_g = wv_pool.tile([128, 2 * DT, 512], FP8, tag="wv")
                    nc.sync.dma_start(
                        out=wv_g[:].rearrange("p k n -> p (k n)"),
                        in_=wv_d.ap()[g],
                    )
                wv_gs[g] = wv_g
                v4s[g] = []

            def v_chunk(g, sts):
                """v4[st] = X @ Wv_quad + bv (+ interleaved 32.0 cols).

                Here X is the stationary side (psum partitions = s rows) and
                Wv the moving side, so the residual pairing flips: instr1
                lhsT=(Xh_d,Xl_d) vs broadcast Wh_d; instr2 strided Xh vs
                strided Wl. d-outer over the st group so each xt subtile is
                consumed as soon as its DMA lands (matters for the startup
                ramp)."""
                wv_g = wv_gs[g]
                wvv = wv_g[:].rearrange("p (d two) n -> p two d n", two=2)
                xv = xt[:].rearrange("p (d two) s -> p two d s", two=2)
                for pair in (list(sts)[:2], list(sts)[2:]):
                    psvs = []
                    for _st in pair:
                        psv = ps_proj.tile([128, 512], F32, tag="ps_proj")
                        psvs.append(psv)
                    for d in range(0, DT, 2):
                        for dd in (d, d + 1):
                            for st, psv in zip(pair, psvs):
                                nc.tensor.matmul(
                                    psv[:],
                                    lhsT=xt[:, 2 * dd : 2 * dd + 2,
                                            st * 128 : (st + 1) * 128],
                                    rhs=wvv[:, 0, dd, :]
                                    .unsqueeze(1)
                                    .to_broadcast([128, 2, 512]),
                                    start=(dd == 0),
                                    stop=False,
                                    perf_mode=DR,
                                )
                        for st, psv in zip(pair, psvs):
                            nc.tensor.matmul(
                                psv[:],
                                lhsT=xv[:, 0, d : d + 2,
                                        st * 128 : (st + 1) * 128],
                                rhs=wvv[:, 1, d : d + 2, :],
                                start=False,
                                stop=(d == DT - 2),
                                perf_mode=DR,
                            )
                    for st, psv in zip(pair, psvs):
                        v4t = v4_pool.tile([128, 520], BF16, tag=f"v4_{st}")
                        nc.vector.tensor_copy(v4t[:], bvq[:, g, :])
                        dst = v4t[:].rearrange("p (q c) -> p q c", q=4)[:, :, 0:128]
                        src = psv[:].rearrange("p (q c) -> p q c", q=4)
                        nc.vector.tensor_add(dst, dst, src)
                        v4s[g].append(v4t)

            def qkv_start(h):
                if h == 0 and _rep == 0:
                    wqk_h = wqk_first
                else:
                    wqk_h = wqk_pool.tile([128, 4 * DT, 128], FP8, tag="wqk")
                    nc.sync.dma_start(
                        out=wqk_h[:].rearrange("p k m -> p (k m)"), in_=wqk_d.ap()[h]
                    )
                qT = qk_pool.tile([128, S], BF16, tag="qT")
                kT = qk_pool.tile([128, S], BF16, tag="kT")
                qks[h] = (qT, kT, wqk_h)

            def qkv_chunk(h, sb):
                """qT/kT columns for s-block sb of head h."""
                qT, kT, wqk_h = qks[h]
                for idx, dest in ((0, qT), (1, kT)):
                    psx = ps_proj.tile([128, 512], F32, tag="ps_proj")
                    resid_mms(
                        psx[:],
                        wqk_h[:, 2 * DT * idx : 2 * DT * (idx + 1), :],
                        sb * 512,
                        512,
                    )
                    nc.vector.tensor_scalar_add(
                        dest[:, sb * 512 : (sb + 1) * 512], psx[:],
                        bqk[:, 8 * idx + h : 8 * idx + h + 1],
                    )

            attn_ps = {}  # (h, qb) -> [(p_tile, off)]

            def attn_scores(h, qb, lo=0, hi=None):
                """Scores + exp (+causal mask) for q-block qb of head h.

                Diagonal k-tiles are trimmed to their live width: tile kt
                covers q_local in [off, 512) with off = max(kt*128-qb*512, 0).
                """
                qT, kT = qks[h][0], qks[h][1]
                n_kt = 4 * qb + 4
                if hi is None:
                    hi = n_kt
                ps = attn_ps.setdefault((h, qb), [])
                for kt in range(lo, hi):
                    r = kt * 128 - qb * 512
                    off = max(r, 0)
                    nw = 512 - off
                    pss = ps_sc.tile([128, 512], F32, tag="ps_sc")
                    nc.tensor.matmul(
                        pss[:, 0:nw],
                        lhsT=kT[:, kt * 128 : (kt + 1) * 128],
                        rhs=qT[:, qb * 512 + off : (qb + 1) * 512],
                        start=True,
                        stop=True,
                    )
                    p_sb = p_pool.tile([128, 512], BF16, tag="p")
                    nc.scalar.activation(
                        p_sb[:, 0:nw], pss[:, 0:nw],
                        mybir.ActivationFunctionType.Exp, scale=SCALE / (WS * WS),
                    )
                    if r >= 0:  # diagonal block: apply causal 0/1 mask
                        nc.vector.tensor_mul(
                            p_sb[:, 0:nw], p_sb[:, 0:nw], mask[:, 384 : 384 + nw]
                        )
                    ps.append((p_sb, off))

            def attn_out(h, qb, split_dma=False):
                """attn @ [v|32], normalize, and store, for q-block qb.

                split_dma: store each j-block as its own DMA (used for the
                very last block so only a small [128,128] store trails the
                final PV chain instead of the whole [512,128] tile)."""
                g, hq = h // 4, h % 4
                v4 = v4s[g]
                ps = attn_ps.pop((h, qb))
                o4 = o_pool.tile([128, 512], F32, tag="o")
                for j in range(4):
                    poj = ps_out.tile([128, 129], F32, tag="po")
                    last_kt = 4 * qb + j  # causality: kt*128 <= qb*512 + j*128
                    for kt in range(last_kt + 1):
                        p_sb, off = ps[kt]
                        nc.tensor.matmul(
                            poj[:],
                            lhsT=p_sb[:, j * 128 - off : j * 128 - off + 128],
                            rhs=v4[kt][:, hq * 130 : hq * 130 + 129],
                            start=(kt == 0),
                            stop=(kt == last_kt),
                        )
                    recip = r_pool.tile([128, 1], F32, tag="recip")
                    nc.vector.reciprocal(recip[:], poj[:, 128:129])
                    nc.vector.tensor_scalar_mul(
                        o4[:, j * 128 : (j + 1) * 128], poj[:, 0:128], recip[:]
                    )
                    if split_dma:
                        nc.sync.dma_start(
                            out=out_d.ap()[
                                qb * 512 + j * 128 : qb * 512 + (j + 1) * 128,
                                h * 128 : (h + 1) * 128,
                            ],
                            in_=o4[:, j * 128 : (j + 1) * 128],
                        )
                if not split_dma:
                    # one DMA per (h, qb): dram [512,128] -> [128 p, 4 j, 128]
                    nc.sync.dma_start(
                        out=out_d.ap()[
                            qb * 512 : (qb + 1) * 512, h * 128 : (h + 1) * 128
                        ].rearrange("(j p) c -> p j c", j=4),
                        in_=o4[:].rearrange("p (j c) -> p j c", j=4),
                    )

            # ---- software-pipelined emission ----
            # plan rows (kind, idx, h_attn): the item's 4 chunks are
            # interleaved with the attention of head h_attn. When h_attn ==
            # idx for a qkv item ("JIT"), chunk sb runs just before the
            # scores that need it (shift-by-one), so the last heads still
            # overlap exp with their own projection matmuls. Weight DMAs for
            # step s+1 are issued mid-step s (the pools are double-buffered).
            plan = [
                ("v", 0, None),
                ("qkv", 0, None),
                ("qkv", 1, 0),
                ("qkv", 2, 1),
                ("qkv", 3, 2),
                ("qkv", 4, 3),
                ("v", 1, 4),
                ("qkv", 5, 5),
                ("qkv", 6, 6),
                ("qkv", 7, 7),
            ]
            started = set()

            def ensure_start(si):
                if si < len(plan) and si not in started:
                    started.add(si)
                    kind, idx, _ = plan[si]
                    if kind == "v":
                        v_start(idx)
                    else:
                        qkv_start(idx)

            def chunks_of(kind, idx):
                if kind == "v":
                    return [lambda i=i: v_chunk(idx, range(4 * i, 4 * i + 4))
                            for i in range(4)]
                return [lambda sb=sb: qkv_chunk(idx, sb) for sb in range(NQB)]

            ensure_start(0)
            _lo = {0: 0, 1: 2, 2: 4, 3: 5}
            for si, (kind, idx, h_attn) in enumerate(plan):
                jit = kind == "qkv" and h_attn == idx
                pe_chunks = chunks_of(kind, idx)
                if si == len(plan) - 1:
                    # final step: no later projection hides this head's exp
                    # burst, so project q-block 3 FIRST ([3,0,1,2]) and dribble
                    # qb3's scores out as their kT blocks appear — the 16-exp
                    # burst then overlaps the step's own projection matmuls
                    # instead of serializing the tail. attn_out(3) runs before
                    # attn_out(2), whose exps are long done, so the PE runs
                    # PV chains back-to-back to the end.
                    h = h_attn
                    pe_chunks[3]()
                    pe_chunks[0]()
                    attn_scores(h, 3, lo=0, hi=4)
                    attn_scores(h, 0)
                    pe_chunks[1]()
                    attn_out(h, 0)
                    attn_scores(h, 3, lo=4, hi=8)
                    attn_scores(h, 1)
                    pe_chunks[2]()
                    attn_out(h, 1)
                    attn_scores(h, 3, lo=8)
                    attn_scores(h, 2)
                    attn_out(h, 3)
                    attn_out(h, 2, split_dma=True)
                    continue
                if jit:
                    pe_chunks[0]()
                for i in range(NQB):
                    if h_attn is not None:
                        attn_scores(h_attn, i, lo=_lo[i])
                    if not jit:
                        pe_chunks[i]()
                    elif i + 1 < NQB:
                        pe_chunks[i + 1]()
                    if i == 1:
                        ensure_start(si + 1)
                    if h_attn is not None:
                        if i + 1 < NQB and _lo[i + 1] > 0:
                            attn_scores(h_attn, i + 1, lo=0, hi=_lo[i + 1])
                        attn_out(h_attn, i)
    nc.compile()
    return nc


def _fp8_split(a):
    """fp32 array -> (hi, lo) fp8 e4m3 with hi + lo ~= a."""
    hi = a.astype(FP8_NP)
    lo = (a - hi.astype(np.float32)).astype(FP8_NP)
    return hi, lo


def make_in_maps(hidden_states, W, b):
    """Host-side sharding: slice/transpose/cast inputs per core."""
    X = np.asarray(hidden_states, dtype=np.float32)
    Wf = np.asarray(W, dtype=np.float32).reshape(D, D, 3) * WS
    bf = np.asarray(b, dtype=np.float32).reshape(D, 3) * WS

    # causal staircase mask: mask[p, c] = 1 if c >= p + 384
    cols = np.arange(896)[None, :]
    rows = np.arange(128)[:, None]
    mask = (cols >= rows + 384).astype(BF16_NP)

    def interleave(hi, lo, m):
        # [DT, 128, m] pair -> [128, 2*DT, m] with (hi_d, lo_d) subtiles
        st = np.stack([hi, lo], axis=1)  # [DT, 2, 128, m]
        return np.ascontiguousarray(st.transpose(2, 0, 1, 3)).reshape(
            128, 2 * DT, m
        )

    in_maps = []
    for c in range(N_CORES):
        bcore, hg = c // 2, c % 2
        dm0 = hg * 1024
        xtf = np.ascontiguousarray(X[bcore].T).reshape(DT, 128, S)
        xh, xl = _fp8_split(xtf)
        # dram layout [2*DT, 128, S] with order (hi_0, lo_0, hi_1, ...)
        xt8 = np.stack([xh, xl], axis=1).reshape(2 * DT, 128, S)

        def w_cols(c0, c1, t, m):
            # cols [c0,c1) within this core's 1024; t: 0=q,1=v,2=k
            wf = Wf[:, dm0 + c0 : dm0 + c1, t].reshape(DT, 128, m)
            hi, lo = _fp8_split(wf)
            return interleave(hi, lo, m)

        wqk = np.stack(
            [
                np.concatenate(
                    [
                        w_cols(h * 128, (h + 1) * 128, 0, 128),
                        w_cols(h * 128, (h + 1) * 128, 2, 128),
                    ],
                    axis=1,
                ).reshape(128, 4 * DT * 128)
                for h in range(8)
            ]
        )
        wv = np.stack(
            [
                w_cols(g * 512, (g + 1) * 512, 1, 512).reshape(128, 2 * DT * 512)
                for g in range(2)
            ]
        )
        bqk = np.concatenate(
            [
                np.ascontiguousarray(bf[dm0 : dm0 + 1024, 0].reshape(8, 128).T),
                np.ascontiguousarray(bf[dm0 : dm0 + 1024, 2].reshape(8, 128).T),
            ],
            axis=1,
        ).astype(np.float32)
        bv = bf[dm0 : dm0 + 1024, 1].reshape(2, 4, 128)
        bvq = np.zeros((2, 128, 520), dtype=BF16_NP)
        for g in range(2):
            for hq in range(4):
                bvq[g, :, hq * 130 : hq * 130 + 128] = bv[g, hq][None, :].astype(
                    BF16_NP
                )
                bvq[g, :, hq * 130 + 128] = BF16_NP(WS)
        in_maps.append(
            {
                "xt": xt8, "wqk": wqk, "wv": wv,
                "bqk": bqk, "bvq": bvq, "mask": mask,
            }
        )
    return in_maps


def gather_out(results):
    out = np.empty((B, S, D), dtype=np.float32)
    for c in range(N_CORES):
        bcore, hg = c // 2, c % 2
        out[bcore][:, hg * 1024 : hg * 1024 + 1024] = results[c]["out"]
    return out


_CACHED_NC = None


def kernel(hidden_states, W, b):
    global _CACHED_NC
    if _CACHED_NC is None:
        _CACHED_NC = build_nc()
    in_maps = make_in_maps(hidden_states, W, b)
    res = run_bass_kernel_spmd(_CACHED_NC, in_maps, core_ids=list(range(N_CORES)))
    return gather_out(res.results)
